# revision 17
# baseline (speedup 1.0000x reference)
"""MixAttention Trainium2 kernel.

Reference computation (B=64, N=384, C=768, H=12, hd=64, Nt=128):
    qkv = x @ W_qkv + b_qkv -> q, k, v per head
    t2t: softmax(q[:, :128] @ k[:, :128].T * 1/8) @ v[:, :128]   (template)
    s2a: softmax(q[:, 128:] @ k.T * 1/8) @ v                     (search)
    out = concat @ W_proj + b_proj

Strategy: pure data-parallel over batch, 8 batches per core on 8 cores, no
collectives. All matmul contractions need channel-major (transposed)
operands; x is transposed once on the host (free vs. NEFF exec time). All
GEMMs run in bf16 (fp32 PSUM accumulation): bf16 hits the PE streaming
roofline (~160 ns per 128x384 matmul) while fp32/float32r lower to multi-
pass matmuls at 2-4x the cost.

Emission is a single fused per-batch pipeline, interleaved at head-pair
granularity so the ACT exp stream (the per-head critical dependency) always
runs ~2 head-pairs ahead of the PE matmuls that consume it:
  per hp in 0..5: q/k projection m-tiles hp, hp+6 (W stationary, xT moving;
  q psum drains on DVE with a free-axis-broadcast bias so ACT stays free
  for the exp stream; K^T zero-padded to K=128 stationaries - K=64 moving
  streams at ~2.7x slower rate, and DoublePixel K=64 measures 2x slower
  than padded K=128, so padding wins); v-projection groups in hp 0-2 (xT
  stationary, W_v moving, DVE stt writes the head-parity-split V
  stationaries); scores + exp for heads 2hp, 2hp+1 into a bank-aligned
  [128, 1024] psum (key-chunk jc at cols 0 / 512 / 768, one exp ACT per
  bank); then deferred PV-normalize groups (lag LAG_PV=8 heads, so every
  deferred PV sees fully-written V stationaries and exp results) and
  deferred output-projection groups of the previous batch pop from work
  queues, giving the PE stream work that does not depend on fresh ACT
  results.

PV per head: 3 matmuls ([v|1]-augmented stationaries, E^T moving, key-chunk
0 spans all 384 query columns, chunks 1-2 accumulate the 256 search
columns). Even heads: psum partitions 0-63 = output, 64 = denominator.
Odd heads: partition 0 = denominator, 64-127 = output ([1|0|v] stationary).
Normalize uses reciprocal_approx_fast (~5x faster than the exact ~6
cycles/element DVE reciprocal; ~51 ULP which is noise at bf16 GEMM
precision). HW-measured constraints honored here: the custom-DVE recip
needs a base-partition-0 source (base-64 reads silently give garbage), so
odd heads recip the psum denominator row directly and even heads first copy
partition 64 down with a single-partition DVE copy (the one cross-partition
DVE form walrus accepts); gpsimd partition_broadcast is SBUF-only with a
32-aligned source; DVE tensor ops are lane-locked with all operands at one
base (0 and 64 both work). The zero/ones constant regions of the K^T pads
and V stationaries live in persistent 2-slot buffers memset once in the
prologue, so the steady-state loop carries no memsets and no SBUF->SBUF
shuffle DMAs.
"""

import contextlib

import numpy as np

B, N, C = 64, 384, 768
H, HD = 12, 64
NT = 128          # template tokens (t_h * t_w * 2)
NCORES = 8
NB = B // NCORES  # batches per core
TOK = NB * N      # tokens per core

_PROGRAM = None

LAG_PV = 8        # heads between scores emission and PV consumption


def _build_program(nbatch, e_bf16=True, loop_reps=1, bufs=None, ablate=(),
                   stagger=True):
    import concourse.mybir as mybir
    import concourse.tile as tile
    from concourse import bacc

    f32 = mybir.dt.float32
    bf16 = mybir.dt.bfloat16
    e_dt = bf16 if e_bf16 else mybir.dt.float32r
    Act = mybir.ActivationFunctionType
    Alu = mybir.AluOpType

    bufs = dict(dict(x=2, qk=2, e=10, xt2=2, o=3, gemm=2, pss=2, pv=2, r=3),
                **(bufs or {}))
    nc = bacc.Bacc("TRN2", target_bir_lowering=False)
    tok = nbatch * N

    xT = nc.dram_tensor("xT", [C, tok], bf16, kind="ExternalInput")
    wqkv = nc.dram_tensor("wqkv", [C, 3 * C], bf16, kind="ExternalInput")
    bqkv = nc.dram_tensor("bqkv", [3 * C], f32, kind="ExternalInput")
    wproj = nc.dram_tensor("wproj", [C, C], bf16, kind="ExternalInput")
    bproj = nc.dram_tensor("bproj", [C], f32, kind="ExternalInput")
    out = nc.dram_tensor("out", [tok, C], f32, kind="ExternalOutput")

    NCH = C // 128  # 6 c-chunks
    state = {}      # b -> dict of live tiles

    with tile.TileContext(nc) as tc:
        with (
            tc.tile_pool(name="wpool", bufs=1) as wpool,
            tc.tile_pool(name="xpool", bufs=bufs["x"]) as xpool,
            tc.tile_pool(name="qkpool", bufs=bufs["qk"]) as qkpool,
            tc.tile_pool(name="epool", bufs=bufs["e"]) as epool,
            tc.tile_pool(name="xt2pool", bufs=bufs["xt2"]) as xt2pool,
            tc.tile_pool(name="opool", bufs=bufs["o"]) as opool,
            tc.tile_pool(name="rpool", bufs=bufs["r"]) as rpool,
            tc.tile_pool(name="pspool", bufs=bufs["gemm"],
                         space="PSUM") as pspool,
            tc.tile_pool(name="pvpool", bufs=bufs["pv"],
                         space="PSUM") as pvpool,
        ):
            # ---- resident weights / constants ----
            w_qk, w_v, w_p = [], [], []
            for ci in range(NCH):
                t = wpool.tile([128, 2 * C], bf16, tag=f"wqk{ci}")
                nc.sync.dma_start(t[:], wqkv[ci * 128:(ci + 1) * 128, 0:2 * C])
                w_qk.append(t)
                t = wpool.tile([128, C], bf16, tag=f"wv{ci}")
                nc.sync.dma_start(t[:], wqkv[ci * 128:(ci + 1) * 128,
                                             2 * C:3 * C])
                w_v.append(t)
                t = wpool.tile([128, C], bf16, tag=f"wp{ci}")
                nc.sync.dma_start(t[:], wproj[ci * 128:(ci + 1) * 128, :])
                w_p.append(t)

            bqk = wpool.tile([128, 2 * C // 128], f32, tag="bqk")
            nc.sync.dma_start(
                bqk[:], bqkv[0:2 * C].rearrange("(m p) -> p m", p=128))
            bv_row = wpool.tile([1, C], f32, tag="bvrow")
            nc.sync.dma_start(bv_row[:],
                              bqkv[2 * C:3 * C].rearrange("(a c) -> a c", a=1))
            bv = wpool.tile([128, C], f32, tag="bv")
            nc.gpsimd.partition_broadcast(bv[:], bv_row[:])
            bp_row = wpool.tile([1, C], f32, tag="bprow")
            nc.sync.dma_start(bp_row[:],
                              bproj[:].rearrange("(a c) -> a c", a=1))
            bp = wpool.tile([128, C], f32, tag="bp")
            nc.gpsimd.partition_broadcast(bp[:], bp_row[:])

            # Persistent 2-slot K^T pads and V stationaries; constant
            # regions memset once here, steady state only writes the
            # varying regions.
            kt_slots = []   # [mt][slot] -> (tA, tB)
            for mt in range(6):
                pair = []
                for s in range(2):
                    tA = wpool.tile([128, N], bf16, tag=f"ktA{mt}_{s}")
                    tB = wpool.tile([128, N], bf16, tag=f"ktB{mt}_{s}")
                    nc.vector.memset(tA[64:128, :], 0.0)
                    nc.vector.memset(tB[0:64, :], 0.0)
                    pair.append((tA, tB))
                kt_slots.append(pair)
            v1e_slots, v1o_slots = [], []
            for tt in range(3):
                es, os_ = [], []
                for s in range(2):
                    te = wpool.tile([128, 6, HD + 1], e_dt, tag=f"v1e{tt}_{s}")
                    nc.vector.memset(te[:, :, HD:HD + 1], 1.0)
                    to = wpool.tile([128, 6, 128], e_dt, tag=f"v1o{tt}_{s}")
                    nc.vector.memset(to[:, :, 0:1], 1.0)
                    nc.vector.memset(to[:, :, 1:HD], 0.0)
                    es.append(te)
                    os_.append(to)
                v1e_slots.append(es)
                v1o_slots.append(os_)

            pv_queue = []    # (b, h) waiting for PV+normalize
            proj_queue = []  # (b, tt) waiting for output projection

            def qk_mtile(b, mt):
                st = state[b]
                xt, off = st["xt"], st["xoff"]
                ps = pspool.tile([128, N], f32, tag="gemm",
                                 name=f"psqk{mt}_{b}")
                for ci in range(NCH):
                    nc.tensor.matmul(
                        ps[:], w_qk[ci][:, mt * 128:(mt + 1) * 128],
                        xt[ci][:, off:off + N],
                        start=(ci == 0), stop=(ci == NCH - 1))
                if mt < 6:
                    t = qkpool.tile([128, N], bf16, tag=f"qk{mt}",
                                    name=f"qk{mt}_{b}")
                    # drain on DVE (free-axis-broadcast bias) to keep ACT
                    # free for the exp stream, the per-head critical path
                    nc.vector.scalar_tensor_tensor(
                        out=t[:], in0=ps[:], scalar=1.0,
                        in1=bqk[:, mt:mt + 1].broadcast_to([128, N]),
                        op0=Alu.mult, op1=Alu.add)
                    st["qk"].append(t)
                else:
                    tA, tB = kt_slots[mt - 6][b % 2]
                    nc.scalar.activation(tA[0:64, :], ps[0:64, :],
                                         Act.Identity,
                                         bias=bqk[0:64, mt:mt + 1], scale=1.0)
                    nc.scalar.activation(tB[64:128, :], ps[64:128, :],
                                         Act.Identity,
                                         bias=bqk[64:128, mt:mt + 1],
                                         scale=1.0)
                    st["kt"].append((tA, tB))

            def v_group(b, tt, half):
                st = state[b]
                xt, off = st["xt"], st["xoff"]
                ps = pspool.tile([128, N], f32, tag="gemm",
                                 name=f"psv{tt}{half}_{b}")
                for ci in range(NCH):
                    nc.tensor.matmul(
                        ps[:], xt[ci][:, off + tt * 128:off + (tt + 1) * 128],
                        w_v[ci][:, half * N:(half + 1) * N],
                        start=(ci == 0), stop=(ci == NCH - 1))
                # psum cols = 6 heads x 64 dims; even heads -> v1e cols 0:64,
                # odd heads -> v1o cols 64:128
                ps3 = ps[:].rearrange("p (a b) -> p a b", b=128)
                bv3 = (bv[:, half * N:(half + 1) * N]
                       .rearrange("p (a b) -> p a b", b=128))
                nc.vector.scalar_tensor_tensor(
                    out=st["v1e"][tt][:, 3 * half:3 * half + 3, 0:HD],
                    in0=ps3[:, :, 0:HD], scalar=1.0, in1=bv3[:, :, 0:HD],
                    op0=Alu.mult, op1=Alu.add)
                nc.vector.scalar_tensor_tensor(
                    out=st["v1o"][tt][:, 3 * half:3 * half + 3, HD:2 * HD],
                    in0=ps3[:, :, HD:2 * HD], scalar=1.0,
                    in1=bv3[:, :, HD:2 * HD],
                    op0=Alu.mult, op1=Alu.add)

            def attn_scores(b, h):
                st = state[b]
                hp, part = divmod(h, 2)
                kt_pad = st["kt"][hp][part]
                qt_t = st["qk"][hp]
                # bank-aligned scores psum/E layout: key-chunk 0 (all 384
                # query cols) at cols 0:384, chunks 1-2 (256 search-query
                # cols each) at 512:768 and 768:1024; cols 384:512 unused.
                ps = pvpool.tile([128, 1024], f32, tag="pss",
                                 bufs=bufs["pss"], name=f"pss{h}_{b}")
                et = epool.tile([128, 1024], e_dt, tag="e", name=f"e{h}_{b}")
                nc.tensor.matmul(ps[:, 0:N], kt_pad[:, 0:128], qt_t[:, 0:N],
                                 start=True, stop=True)
                nc.tensor.matmul(ps[:, 512:768], kt_pad[:, 128:256],
                                 qt_t[:, 128:N], start=True, stop=True)
                nc.tensor.matmul(ps[:, 768:1024], kt_pad[:, 256:384],
                                 qt_t[:, 128:N], start=True, stop=True)
                nc.scalar.activation(et[:, 0:N], ps[:, 0:N],
                                     Act.Exp, bias=0.0, scale=0.125)
                nc.scalar.activation(et[:, 512:1024], ps[:, 512:1024],
                                     Act.Exp, bias=0.0, scale=0.125)
                st["e"][h] = et

            def attn_pv(b, h):
                st = state[b]
                xt2 = st["xt2"]
                et = st["e"].pop(h)
                even = h % 2 == 0
                v1 = st["v1e"] if even else st["v1o"]
                pv = pvpool.tile([128, N], f32, tag="pspv",
                                 name=f"pv{h}_{b}")
                np_ = HD + 1 if even else 128
                nc.tensor.matmul(pv[0:np_, 0:N], v1[0][:, h // 2, :],
                                 et[:, 0:N], start=True, stop=False,
                                 skip_group_check=True)
                nc.tensor.matmul(pv[0:np_, 128:N], v1[1][:, h // 2, :],
                                 et[:, 512:768], start=False, stop=False,
                                 skip_group_check=True)
                nc.tensor.matmul(pv[0:np_, 128:N], v1[2][:, h // 2, :],
                                 et[:, 768:1024], start=False, stop=True,
                                 skip_group_check=True)
                rr = rpool.tile([1, N], f32, tag="rr", name=f"rr{h}_{b}")
                if even:
                    dd = rpool.tile([1, N], f32, tag="dd", name=f"dd{h}_{b}")
                    nc.vector.tensor_copy(dd[:], pv[HD:HD + 1, :])
                    nc.vector.reciprocal_approx_fast(rr[:], dd[:])
                    brc = rpool.tile([HD, N], f32, tag="brc",
                                     name=f"brc{h}_{b}")
                    nc.gpsimd.partition_broadcast(brc[:], rr[:])
                    nc.vector.tensor_mul(xt2[h // 2][0:HD, :], pv[0:HD, :],
                                         brc[:])
                else:
                    nc.vector.reciprocal_approx_fast(rr[:], pv[0:1, :])
                    brc = rpool.tile([128, N], f32, tag="brcf",
                                     name=f"brc{h}_{b}")
                    nc.gpsimd.partition_broadcast(brc[:], rr[:])
                    nc.vector.tensor_mul(xt2[h // 2][HD:128, :],
                                         pv[HD:128, :], brc[HD:128, :])

            def proj_group(b, tt):
                st = state[b]
                xt2 = st["xt2"]
                ot = opool.tile([128, C], f32, tag="osb", name=f"o{tt}_{b}")
                for half in range(2):
                    ps = pspool.tile([128, N], f32, tag="gemm",
                                     name=f"pso{tt}{half}_{b}")
                    for ci in range(NCH):
                        nc.tensor.matmul(
                            ps[:], xt2[ci][:, tt * 128:(tt + 1) * 128],
                            w_p[ci][:, half * N:(half + 1) * N],
                            start=(ci == 0), stop=(ci == NCH - 1))
                    nc.vector.scalar_tensor_tensor(
                        out=ot[:, half * N:(half + 1) * N], in0=ps[:],
                        scalar=1.0, in1=bp[:, half * N:(half + 1) * N],
                        op0=Alu.mult, op1=Alu.add)
                nc.sync.dma_start(
                    out[(b * 3 + tt) * 128:(b * 3 + tt + 1) * 128, :], ot[:])
                st["live"] -= 1
                if st["live"] == 0:
                    del state[b]

            def drain(pv_lag=0, proj_max=2):
                while len(pv_queue) > pv_lag:
                    attn_pv(*pv_queue.pop(0))
                n = 0
                while proj_queue and n < proj_max:
                    b, tt = proj_queue[0]
                    if any(q[0] == b for q in pv_queue):
                        break  # that batch's xt2 not fully emitted yet
                    proj_group(*proj_queue.pop(0))
                    n += 1

            def drain_final():
                # interleave PV chains with PE-heavy proj groups so the
                # trailing ACT/DVE latency hides under matmuls
                while pv_queue or proj_queue:
                    for _ in range(2):
                        if pv_queue:
                            attn_pv(*pv_queue.pop(0))
                    if proj_queue:
                        b, tt = proj_queue[0]
                        if not any(q[0] == b for q in pv_queue):
                            proj_group(*proj_queue.pop(0))

            def fused(b):
                st = state[b] = {"qk": [], "kt": [], "e": {}, "live": 3}
                if b % 2 == 0:
                    xt = []
                    for ci in range(NCH):
                        t = xpool.tile([128, 2 * N], bf16, tag=f"xt{ci}",
                                       name=f"xt{ci}_{b}")
                        w = min(2 * N, tok - b * N)
                        nc.sync.dma_start(
                            t[:, 0:w],
                            xT[ci * 128:(ci + 1) * 128, b * N:b * N + w])
                        xt.append(t)
                    st["xt"], st["xoff"] = xt, 0
                else:
                    st["xt"], st["xoff"] = state[b - 1]["xt"], N
                st["v1e"] = [v1e_slots[tt][b % 2] for tt in range(3)]
                st["v1o"] = [v1o_slots[tt][b % 2] for tt in range(3)]
                st["xt2"] = [
                    xt2pool.tile([128, N], bf16, tag=f"xt2{ci}",
                                 name=f"xt2{ci}_{b}")
                    for ci in range(NCH)]
                # v-projection groups all land in hp 0-2 so that every PV
                # popped at hp>=3 (lag 6 heads) sees fully-written V
                # stationaries; output projection of the previous batch pops
                # at hp>=3, after that batch's last PV popped at hp2.
                for hp in range(6):
                    qk_mtile(b, hp)
                    qk_mtile(b, hp + 6)
                    if hp < 3:
                        v_group(b, hp, 0)
                        v_group(b, hp, 1)
                    attn_scores(b, 2 * hp)
                    pv_queue.append((b, 2 * hp))
                    attn_scores(b, 2 * hp + 1)
                    pv_queue.append((b, 2 * hp + 1))
                    drain(pv_lag=LAG_PV, proj_max=0 if hp < 3 else 2)
                proj_queue.extend((b, tt) for tt in range(3))

            loop_cm = (tc.For_i(0, loop_reps, 1) if loop_reps > 1
                       else contextlib.nullcontext())
            with loop_cm:
                for b in range(nbatch):
                    fused(b)
                drain_final()
    nc.compile()
    return nc


def _get_program():
    global _PROGRAM
    if _PROGRAM is None:
        _PROGRAM = _build_program(NB)
    return _PROGRAM


def make_in_maps(x, W_qkv, b_qkv, W_proj, b_proj):
    import ml_dtypes
    bf = ml_dtypes.bfloat16
    x = np.asarray(x, dtype=np.float32)
    W_qkv = np.asarray(W_qkv, dtype=np.float32).astype(bf)
    b_qkv = np.asarray(b_qkv, dtype=np.float32)
    W_proj = np.asarray(W_proj, dtype=np.float32).astype(bf)
    b_proj = np.asarray(b_proj, dtype=np.float32)
    in_maps = []
    for i in range(NCORES):
        xc = x[i * NB:(i + 1) * NB].reshape(TOK, C)
        in_maps.append({
            "xT": np.ascontiguousarray(xc.T).astype(bf),
            "wqkv": W_qkv, "bqkv": b_qkv,
            "wproj": W_proj, "bproj": b_proj,
        })
    return in_maps


def kernel(x, W_qkv, b_qkv, W_proj, b_proj, t_h, t_w, s_h, s_w):
    from concourse.bass_utils import run_bass_kernel_spmd

    x = np.asarray(x, dtype=np.float32)
    assert x.shape == (B, N, C)
    assert int(t_h) * int(t_w) * 2 == NT
    assert int(s_h) * int(s_w) == N - NT

    nc = _get_program()
    in_maps = make_in_maps(x, W_qkv, b_qkv, W_proj, b_proj)
    res = run_bass_kernel_spmd(nc, in_maps, core_ids=list(range(NCORES)))
    return np.concatenate(
        [r["out"].reshape(NB, N, C) for r in res.results], axis=0)


# revision 18
# speedup vs baseline: 1.0021x; 1.0021x over previous
"""MixAttention Trainium2 kernel.

Reference computation (B=64, N=384, C=768, H=12, hd=64, Nt=128):
    qkv = x @ W_qkv + b_qkv -> q, k, v per head
    t2t: softmax(q[:, :128] @ k[:, :128].T * 1/8) @ v[:, :128]   (template)
    s2a: softmax(q[:, 128:] @ k.T * 1/8) @ v                     (search)
    out = concat @ W_proj + b_proj

Strategy: pure data-parallel over batch, 8 batches per core on 8 cores, no
collectives. All matmul contractions need channel-major (transposed)
operands; x is transposed once on the host (free vs. NEFF exec time). All
GEMMs run in bf16 (fp32 PSUM accumulation): bf16 hits the PE streaming
roofline (~160 ns per 128x384 matmul) while fp32/float32r lower to multi-
pass matmuls at 2-4x the cost.

Emission is a single fused per-batch pipeline, interleaved at head-pair
granularity so the ACT exp stream (the per-head critical dependency) always
runs ~2 head-pairs ahead of the PE matmuls that consume it:
  per hp in 0..5: q/k projection m-tiles hp, hp+6 (W stationary, xT moving;
  q psum drains on DVE with a free-axis-broadcast bias so ACT stays free
  for the exp stream; K^T zero-padded to K=128 stationaries - K=64 moving
  streams at ~2.7x slower rate, and DoublePixel K=64 measures 2x slower
  than padded K=128, so padding wins); v-projection groups in hp 0-2 (xT
  stationary, W_v moving, DVE stt writes the head-parity-split V
  stationaries); scores + exp for heads 2hp, 2hp+1 into a bank-aligned
  [128, 1024] psum (key-chunk jc at cols 0 / 512 / 768, one exp ACT per
  bank); then deferred PV-normalize groups (lag LAG_PV=8 heads, so every
  deferred PV sees fully-written V stationaries and exp results) and
  deferred output-projection groups of the previous batch pop from work
  queues, giving the PE stream work that does not depend on fresh ACT
  results.

PV per head: 3 matmuls ([v|1]-augmented stationaries, E^T moving, key-chunk
0 spans all 384 query columns, chunks 1-2 accumulate the 256 search
columns). Even heads: psum partitions 0-63 = output, 64 = denominator.
Odd heads: partition 0 = denominator, 64-127 = output ([1|0|v] stationary).
Normalize uses reciprocal_approx_fast (~5x faster than the exact ~6
cycles/element DVE reciprocal; ~51 ULP which is noise at bf16 GEMM
precision). HW-measured constraints honored here: the custom-DVE recip
needs a base-partition-0 source (base-64 reads silently give garbage), so
odd heads recip the psum denominator row directly and even heads first copy
partition 64 down with a single-partition DVE copy (the one cross-partition
DVE form walrus accepts); gpsimd partition_broadcast is SBUF-only with a
32-aligned source; DVE tensor ops are lane-locked with all operands at one
base (0 and 64 both work). The zero/ones constant regions of the K^T pads
and V stationaries live in persistent 2-slot buffers memset once in the
prologue, so the steady-state loop carries no memsets and no SBUF->SBUF
shuffle DMAs.
"""

import contextlib

import numpy as np

B, N, C = 64, 384, 768
H, HD = 12, 64
NT = 128          # template tokens (t_h * t_w * 2)
NCORES = 8
NB = B // NCORES  # batches per core
TOK = NB * N      # tokens per core

_PROGRAM = None

LAG_PV = 8        # heads between scores emission and PV consumption


def _build_program(nbatch, e_bf16=True, loop_reps=1, bufs=None, ablate=(),
                   stagger=True):
    import concourse.mybir as mybir
    import concourse.tile as tile
    from concourse import bacc

    f32 = mybir.dt.float32
    bf16 = mybir.dt.bfloat16
    e_dt = bf16 if e_bf16 else mybir.dt.float32r
    Act = mybir.ActivationFunctionType
    Alu = mybir.AluOpType

    bufs = dict(dict(x=2, qk=2, e=10, xt2=2, o=3, gemm=2, pss=2, pv=2, r=3),
                **(bufs or {}))
    nc = bacc.Bacc("TRN2", target_bir_lowering=False)
    tok = nbatch * N

    xT = nc.dram_tensor("xT", [C, tok], bf16, kind="ExternalInput")
    wqkv = nc.dram_tensor("wqkv", [C, 3 * C], bf16, kind="ExternalInput")
    bqkv = nc.dram_tensor("bqkv", [3 * C], f32, kind="ExternalInput")
    wproj = nc.dram_tensor("wproj", [C, C], bf16, kind="ExternalInput")
    bproj = nc.dram_tensor("bproj", [C], f32, kind="ExternalInput")
    out = nc.dram_tensor("out", [tok, C], f32, kind="ExternalOutput")

    NCH = C // 128  # 6 c-chunks
    state = {}      # b -> dict of live tiles

    with tile.TileContext(nc) as tc:
        with (
            tc.tile_pool(name="wpool", bufs=1) as wpool,
            tc.tile_pool(name="xpool", bufs=bufs["x"]) as xpool,
            tc.tile_pool(name="qkpool", bufs=bufs["qk"]) as qkpool,
            tc.tile_pool(name="epool", bufs=bufs["e"]) as epool,
            tc.tile_pool(name="xt2pool", bufs=bufs["xt2"]) as xt2pool,
            tc.tile_pool(name="opool", bufs=bufs["o"]) as opool,
            tc.tile_pool(name="rpool", bufs=bufs["r"]) as rpool,
            tc.tile_pool(name="pspool", bufs=bufs["gemm"],
                         space="PSUM") as pspool,
            tc.tile_pool(name="pvpool", bufs=bufs["pv"],
                         space="PSUM") as pvpool,
        ):
            # ---- resident weights / constants ----
            w_qk, w_v, w_p = [], [], []
            for ci in range(NCH):
                t = wpool.tile([128, 2 * C], bf16, tag=f"wqk{ci}")
                nc.sync.dma_start(t[:], wqkv[ci * 128:(ci + 1) * 128, 0:2 * C])
                w_qk.append(t)
                t = wpool.tile([128, C], bf16, tag=f"wv{ci}")
                nc.sync.dma_start(t[:], wqkv[ci * 128:(ci + 1) * 128,
                                             2 * C:3 * C])
                w_v.append(t)
                t = wpool.tile([128, C], bf16, tag=f"wp{ci}")
                nc.sync.dma_start(t[:], wproj[ci * 128:(ci + 1) * 128, :])
                w_p.append(t)

            bqk = wpool.tile([128, 2 * C // 128], f32, tag="bqk")
            nc.sync.dma_start(
                bqk[:], bqkv[0:2 * C].rearrange("(m p) -> p m", p=128))
            bv_row = wpool.tile([1, C], f32, tag="bvrow")
            nc.sync.dma_start(bv_row[:],
                              bqkv[2 * C:3 * C].rearrange("(a c) -> a c", a=1))
            bv = wpool.tile([128, C], f32, tag="bv")
            nc.gpsimd.partition_broadcast(bv[:], bv_row[:])
            bp_row = wpool.tile([1, C], f32, tag="bprow")
            nc.sync.dma_start(bp_row[:],
                              bproj[:].rearrange("(a c) -> a c", a=1))
            bp = wpool.tile([128, C], f32, tag="bp")
            nc.gpsimd.partition_broadcast(bp[:], bp_row[:])

            # Persistent 2-slot K^T pads and V stationaries; constant
            # regions memset once here, steady state only writes the
            # varying regions.
            kt_slots = []   # [mt][slot] -> (tA, tB)
            for mt in range(6):
                pair = []
                for s in range(2):
                    tA = wpool.tile([128, N], bf16, tag=f"ktA{mt}_{s}")
                    tB = wpool.tile([128, N], bf16, tag=f"ktB{mt}_{s}")
                    nc.vector.memset(tA[64:128, :], 0.0)
                    nc.vector.memset(tB[0:64, :], 0.0)
                    pair.append((tA, tB))
                kt_slots.append(pair)
            v1e_slots, v1o_slots = [], []
            for tt in range(3):
                es, os_ = [], []
                for s in range(2):
                    te = wpool.tile([128, 6, HD + 1], e_dt, tag=f"v1e{tt}_{s}")
                    nc.vector.memset(te[:, :, HD:HD + 1], 1.0)
                    to = wpool.tile([128, 6, 128], e_dt, tag=f"v1o{tt}_{s}")
                    nc.vector.memset(to[:, :, 0:1], 1.0)
                    nc.vector.memset(to[:, :, 1:HD], 0.0)
                    es.append(te)
                    os_.append(to)
                v1e_slots.append(es)
                v1o_slots.append(os_)

            pv_queue = []    # (b, h) waiting for PV+normalize
            proj_queue = []  # (b, tt) waiting for output projection

            def qk_mtile(b, mt):
                st = state[b]
                xt, off = st["xt"], st["xoff"]
                ps = pspool.tile([128, N], f32, tag="gemm",
                                 name=f"psqk{mt}_{b}")
                for ci in range(NCH):
                    nc.tensor.matmul(
                        ps[:], w_qk[ci][:, mt * 128:(mt + 1) * 128],
                        xt[ci][:, off:off + N],
                        start=(ci == 0), stop=(ci == NCH - 1))
                if mt < 6:
                    t = qkpool.tile([128, N], bf16, tag=f"qk{mt}",
                                    name=f"qk{mt}_{b}")
                    # drain on DVE (free-axis-broadcast bias) to keep ACT
                    # free for the exp stream, the per-head critical path
                    nc.vector.scalar_tensor_tensor(
                        out=t[:], in0=ps[:], scalar=1.0,
                        in1=bqk[:, mt:mt + 1].broadcast_to([128, N]),
                        op0=Alu.mult, op1=Alu.add)
                    st["qk"].append(t)
                else:
                    tA, tB = kt_slots[mt - 6][b % 2]
                    nc.scalar.activation(tA[0:64, :], ps[0:64, :],
                                         Act.Identity,
                                         bias=bqk[0:64, mt:mt + 1], scale=1.0)
                    nc.scalar.activation(tB[64:128, :], ps[64:128, :],
                                         Act.Identity,
                                         bias=bqk[64:128, mt:mt + 1],
                                         scale=1.0)
                    st["kt"].append((tA, tB))

            def v_group(b, tt, half):
                st = state[b]
                xt, off = st["xt"], st["xoff"]
                ps = pspool.tile([128, N], f32, tag="gemm",
                                 name=f"psv{tt}{half}_{b}")
                for ci in range(NCH):
                    nc.tensor.matmul(
                        ps[:], xt[ci][:, off + tt * 128:off + (tt + 1) * 128],
                        w_v[ci][:, half * N:(half + 1) * N],
                        start=(ci == 0), stop=(ci == NCH - 1))
                # psum cols = 6 heads x 64 dims; even heads -> v1e cols 0:64,
                # odd heads -> v1o cols 64:128
                ps3 = ps[:].rearrange("p (a b) -> p a b", b=128)
                bv3 = (bv[:, half * N:(half + 1) * N]
                       .rearrange("p (a b) -> p a b", b=128))
                nc.vector.scalar_tensor_tensor(
                    out=st["v1e"][tt][:, 3 * half:3 * half + 3, 0:HD],
                    in0=ps3[:, :, 0:HD], scalar=1.0, in1=bv3[:, :, 0:HD],
                    op0=Alu.mult, op1=Alu.add)
                nc.vector.scalar_tensor_tensor(
                    out=st["v1o"][tt][:, 3 * half:3 * half + 3, HD:2 * HD],
                    in0=ps3[:, :, HD:2 * HD], scalar=1.0,
                    in1=bv3[:, :, HD:2 * HD],
                    op0=Alu.mult, op1=Alu.add)

            def attn_scores(b, h):
                st = state[b]
                hp, part = divmod(h, 2)
                kt_pad = st["kt"][hp][part]
                qt_t = st["qk"][hp]
                # bank-aligned scores psum/E layout: key-chunk 0 (all 384
                # query cols) at cols 0:384, chunks 1-2 (256 search-query
                # cols each) at 512:768 and 768:1024; cols 384:512 unused.
                ps = pvpool.tile([128, 1024], f32, tag="pss",
                                 bufs=bufs["pss"], name=f"pss{h}_{b}")
                et = epool.tile([128, 1024], e_dt, tag="e", name=f"e{h}_{b}")
                nc.tensor.matmul(ps[:, 0:N], kt_pad[:, 0:128], qt_t[:, 0:N],
                                 start=True, stop=True)
                nc.tensor.matmul(ps[:, 512:768], kt_pad[:, 128:256],
                                 qt_t[:, 128:N], start=True, stop=True)
                nc.tensor.matmul(ps[:, 768:1024], kt_pad[:, 256:384],
                                 qt_t[:, 128:N], start=True, stop=True)
                nc.scalar.activation(et[:, 0:N], ps[:, 0:N],
                                     Act.Exp, bias=0.0, scale=0.125)
                nc.scalar.activation(et[:, 512:1024], ps[:, 512:1024],
                                     Act.Exp, bias=0.0, scale=0.125)
                st["e"][h] = et

            def attn_pv(b, h):
                st = state[b]
                xt2 = st["xt2"]
                et = st["e"].pop(h)
                even = h % 2 == 0
                v1 = st["v1e"] if even else st["v1o"]
                pv = pvpool.tile([128, N], f32, tag="pspv",
                                 name=f"pv{h}_{b}")
                np_ = HD + 1 if even else 128
                nc.tensor.matmul(pv[0:np_, 0:N], v1[0][:, h // 2, :],
                                 et[:, 0:N], start=True, stop=False,
                                 skip_group_check=True)
                nc.tensor.matmul(pv[0:np_, 128:N], v1[1][:, h // 2, :],
                                 et[:, 512:768], start=False, stop=False,
                                 skip_group_check=True)
                nc.tensor.matmul(pv[0:np_, 128:N], v1[2][:, h // 2, :],
                                 et[:, 768:1024], start=False, stop=True,
                                 skip_group_check=True)
                rr = rpool.tile([1, N], f32, tag="rr", name=f"rr{h}_{b}")
                if even:
                    dd = rpool.tile([1, N], f32, tag="dd", name=f"dd{h}_{b}")
                    nc.vector.tensor_copy(dd[:], pv[HD:HD + 1, :])
                    nc.vector.reciprocal_approx_fast(rr[:], dd[:])
                    brc = rpool.tile([HD, N], f32, tag="brc",
                                     name=f"brc{h}_{b}")
                    nc.gpsimd.partition_broadcast(brc[:], rr[:])
                    nc.vector.tensor_mul(xt2[h // 2][0:HD, :], pv[0:HD, :],
                                         brc[:])
                else:
                    nc.vector.reciprocal_approx_fast(rr[:], pv[0:1, :])
                    brc = rpool.tile([128, N], f32, tag="brcf",
                                     name=f"brc{h}_{b}")
                    nc.gpsimd.partition_broadcast(brc[:], rr[:])
                    nc.vector.tensor_mul(xt2[h // 2][HD:128, :],
                                         pv[HD:128, :], brc[HD:128, :])

            def proj_group(b, tt):
                st = state[b]
                xt2 = st["xt2"]
                ot = opool.tile([128, C], f32, tag="osb", name=f"o{tt}_{b}")
                for half in range(2):
                    ps = pspool.tile([128, N], f32, tag="gemm",
                                     name=f"pso{tt}{half}_{b}")
                    for ci in range(NCH):
                        nc.tensor.matmul(
                            ps[:], xt2[ci][:, tt * 128:(tt + 1) * 128],
                            w_p[ci][:, half * N:(half + 1) * N],
                            start=(ci == 0), stop=(ci == NCH - 1))
                    nc.vector.scalar_tensor_tensor(
                        out=ot[:, half * N:(half + 1) * N], in0=ps[:],
                        scalar=1.0, in1=bp[:, half * N:(half + 1) * N],
                        op0=Alu.mult, op1=Alu.add)
                nc.sync.dma_start(
                    out[(b * 3 + tt) * 128:(b * 3 + tt + 1) * 128, :], ot[:])
                st["live"] -= 1
                if st["live"] == 0:
                    del state[b]

            def drain(pv_lag=0, proj_max=2):
                while len(pv_queue) > pv_lag:
                    attn_pv(*pv_queue.pop(0))
                n = 0
                while proj_queue and n < proj_max:
                    b, tt = proj_queue[0]
                    if any(q[0] == b for q in pv_queue):
                        break  # that batch's xt2 not fully emitted yet
                    proj_group(*proj_queue.pop(0))
                    n += 1

            def drain_final():
                # interleave PV chains with PE-heavy proj groups so the
                # trailing ACT/DVE latency hides under matmuls
                while pv_queue or proj_queue:
                    for _ in range(2):
                        if pv_queue:
                            attn_pv(*pv_queue.pop(0))
                    if proj_queue:
                        b, tt = proj_queue[0]
                        if not any(q[0] == b for q in pv_queue):
                            proj_group(*proj_queue.pop(0))

            xt_cache = {}  # even batch index -> pair xT tiles

            def load_xt(pb):
                xt = []
                for ci in range(NCH):
                    t = xpool.tile([128, 2 * N], bf16, tag=f"xt{ci}",
                                   name=f"xt{ci}_{pb}")
                    w = min(2 * N, tok - pb * N)
                    nc.sync.dma_start(
                        t[:, 0:w],
                        xT[ci * 128:(ci + 1) * 128, pb * N:pb * N + w])
                    xt.append(t)
                xt_cache[pb] = xt

            def fused(b):
                st = state[b] = {"qk": [], "kt": [], "e": {}, "live": 3}
                if b % 2 == 0:
                    if b not in xt_cache:
                        load_xt(b)
                    st["xt"], st["xoff"] = xt_cache[b], 0
                else:
                    st["xt"], st["xoff"] = xt_cache[b - 1], N
                    if b + 1 < nbatch:
                        # prefetch the next pair's xT a full batch early so
                        # the transfer hides under this batch's compute
                        load_xt(b + 1)
                st["v1e"] = [v1e_slots[tt][b % 2] for tt in range(3)]
                st["v1o"] = [v1o_slots[tt][b % 2] for tt in range(3)]
                st["xt2"] = [
                    xt2pool.tile([128, N], bf16, tag=f"xt2{ci}",
                                 name=f"xt2{ci}_{b}")
                    for ci in range(NCH)]
                # v-projection groups all land in hp 0-2 so that every PV
                # popped at hp>=3 (lag 6 heads) sees fully-written V
                # stationaries; output projection of the previous batch pops
                # at hp>=3, after that batch's last PV popped at hp2.
                for hp in range(6):
                    qk_mtile(b, hp)
                    qk_mtile(b, hp + 6)
                    if hp < 3:
                        v_group(b, hp, 0)
                        v_group(b, hp, 1)
                    attn_scores(b, 2 * hp)
                    pv_queue.append((b, 2 * hp))
                    attn_scores(b, 2 * hp + 1)
                    pv_queue.append((b, 2 * hp + 1))
                    drain(pv_lag=LAG_PV, proj_max=0 if hp < 3 else 2)
                proj_queue.extend((b, tt) for tt in range(3))

            loop_cm = (tc.For_i(0, loop_reps, 1) if loop_reps > 1
                       else contextlib.nullcontext())
            with loop_cm:
                for b in range(nbatch):
                    fused(b)
                drain_final()
    nc.compile()
    return nc


def _get_program():
    global _PROGRAM
    if _PROGRAM is None:
        _PROGRAM = _build_program(NB)
    return _PROGRAM


def make_in_maps(x, W_qkv, b_qkv, W_proj, b_proj):
    import ml_dtypes
    bf = ml_dtypes.bfloat16
    x = np.asarray(x, dtype=np.float32)
    W_qkv = np.asarray(W_qkv, dtype=np.float32).astype(bf)
    b_qkv = np.asarray(b_qkv, dtype=np.float32)
    W_proj = np.asarray(W_proj, dtype=np.float32).astype(bf)
    b_proj = np.asarray(b_proj, dtype=np.float32)
    in_maps = []
    for i in range(NCORES):
        xc = x[i * NB:(i + 1) * NB].reshape(TOK, C)
        in_maps.append({
            "xT": np.ascontiguousarray(xc.T).astype(bf),
            "wqkv": W_qkv, "bqkv": b_qkv,
            "wproj": W_proj, "bproj": b_proj,
        })
    return in_maps


def kernel(x, W_qkv, b_qkv, W_proj, b_proj, t_h, t_w, s_h, s_w):
    from concourse.bass_utils import run_bass_kernel_spmd

    x = np.asarray(x, dtype=np.float32)
    assert x.shape == (B, N, C)
    assert int(t_h) * int(t_w) * 2 == NT
    assert int(s_h) * int(s_w) == N - NT

    nc = _get_program()
    in_maps = make_in_maps(x, W_qkv, b_qkv, W_proj, b_proj)
    res = run_bass_kernel_spmd(nc, in_maps, core_ids=list(range(NCORES)))
    return np.concatenate(
        [r["out"].reshape(NB, N, C) for r in res.results], axis=0)


# revision 20
# speedup vs baseline: 1.0287x; 1.0266x over previous
"""MixAttention Trainium2 kernel.

Reference computation (B=64, N=384, C=768, H=12, hd=64, Nt=128):
    qkv = x @ W_qkv + b_qkv -> q, k, v per head
    t2t: softmax(q[:, :128] @ k[:, :128].T * 1/8) @ v[:, :128]   (template)
    s2a: softmax(q[:, 128:] @ k.T * 1/8) @ v                     (search)
    out = concat @ W_proj + b_proj

Strategy: pure data-parallel over batch, 8 batches per core on 8 cores, no
collectives. All matmul contractions need channel-major (transposed)
operands; x is transposed once on the host (free vs. NEFF exec time). All
GEMMs run in bf16 (fp32 PSUM accumulation): bf16 hits the PE streaming
roofline (~160 ns per 128x384 matmul) while fp32/float32r lower to multi-
pass matmuls at 2-4x the cost.

Emission is a single fused per-batch pipeline, interleaved at head-pair
granularity so the ACT exp stream (the per-head critical dependency) always
runs ~2 head-pairs ahead of the PE matmuls that consume it:
  per hp in 0..5: q/k projection m-tiles hp, hp+6 (W stationary, xT moving;
  q psum drains on DVE with a free-axis-broadcast bias so ACT stays free
  for the exp stream; K^T zero-padded to K=128 stationaries - K=64 moving
  streams at ~2.7x slower rate, and DoublePixel K=64 measures 2x slower
  than padded K=128, so padding wins); v-projection groups in hp 0-2 (xT
  stationary, W_v moving, DVE stt writes the head-parity-split V
  stationaries); scores + exp for heads 2hp, 2hp+1 into a bank-aligned
  [128, 1024] psum (key-chunk jc at cols 0 / 512 / 768, one exp ACT per
  bank); then deferred PV-normalize groups (lag LAG_PV=8 heads, so every
  deferred PV sees fully-written V stationaries and exp results) and
  deferred output-projection groups of the previous batch pop from work
  queues, giving the PE stream work that does not depend on fresh ACT
  results.

PV per head: 3 matmuls ([v|1]-augmented stationaries, E^T moving, key-chunk
0 spans all 384 query columns, chunks 1-2 accumulate the 256 search
columns). Even heads: psum partitions 0-63 = output, 64 = denominator.
Odd heads: partition 0 = denominator, 64-127 = output ([1|0|v] stationary).
Normalize uses reciprocal_approx_fast (~5x faster than the exact ~6
cycles/element DVE reciprocal; ~51 ULP which is noise at bf16 GEMM
precision). HW-measured constraints honored here: the custom-DVE recip
needs a base-partition-0 source (base-64 reads silently give garbage), so
odd heads recip the psum denominator row directly and even heads first copy
partition 64 down with a single-partition DVE copy (the one cross-partition
DVE form walrus accepts); gpsimd partition_broadcast is SBUF-only with a
32-aligned source; DVE tensor ops are lane-locked with all operands at one
base (0 and 64 both work). The zero/ones constant regions of the K^T pads
and V stationaries live in persistent 2-slot buffers memset once in the
prologue, so the steady-state loop carries no memsets and no SBUF->SBUF
shuffle DMAs.
"""

import contextlib

import numpy as np

B, N, C = 64, 384, 768
H, HD = 12, 64
NT = 128          # template tokens (t_h * t_w * 2)
NCORES = 8
NB = B // NCORES  # batches per core
TOK = NB * N      # tokens per core

_PROGRAM = None

LAG_PV = 8        # heads between scores emission and PV consumption


def _build_program(nbatch, e_bf16=True, loop_reps=1, bufs=None, ablate=(),
                   stagger=True):
    import concourse.mybir as mybir
    import concourse.tile as tile
    from concourse import bacc

    f32 = mybir.dt.float32
    bf16 = mybir.dt.bfloat16
    e_dt = bf16 if e_bf16 else mybir.dt.float32r
    Act = mybir.ActivationFunctionType
    Alu = mybir.AluOpType

    bufs = dict(dict(x=2, qk=2, e=10, xt2=2, o=3, gemm=2, pss=2, pv=2, r=3),
                **(bufs or {}))
    nc = bacc.Bacc("TRN2", target_bir_lowering=False)
    tok = nbatch * N

    xT = nc.dram_tensor("xT", [C, tok], bf16, kind="ExternalInput")
    wqkv = nc.dram_tensor("wqkv", [C, 3 * C], bf16, kind="ExternalInput")
    bqkv = nc.dram_tensor("bqkv", [3 * C], f32, kind="ExternalInput")
    wproj = nc.dram_tensor("wproj", [C, C], bf16, kind="ExternalInput")
    bproj = nc.dram_tensor("bproj", [C], f32, kind="ExternalInput")
    out = nc.dram_tensor("out", [tok, C], f32, kind="ExternalOutput")

    NCH = C // 128  # 6 c-chunks
    state = {}      # b -> dict of live tiles

    with tile.TileContext(nc) as tc:
        with (
            tc.tile_pool(name="wpool", bufs=1) as wpool,
            tc.tile_pool(name="xpool", bufs=bufs["x"]) as xpool,
            tc.tile_pool(name="qkpool", bufs=bufs["qk"]) as qkpool,
            tc.tile_pool(name="epool", bufs=bufs["e"]) as epool,
            tc.tile_pool(name="xt2pool", bufs=bufs["xt2"]) as xt2pool,
            tc.tile_pool(name="opool", bufs=bufs["o"]) as opool,
            tc.tile_pool(name="rpool", bufs=bufs["r"]) as rpool,
            tc.tile_pool(name="pspool", bufs=bufs["gemm"],
                         space="PSUM") as pspool,
            tc.tile_pool(name="pvpool", bufs=bufs["pv"],
                         space="PSUM") as pvpool,
        ):
            # ---- resident weights / constants ----
            w_qk, w_v, w_p = [], [], []
            for ci in range(NCH):
                t = wpool.tile([128, 2 * C], bf16, tag=f"wqk{ci}")
                nc.sync.dma_start(t[:], wqkv[ci * 128:(ci + 1) * 128, 0:2 * C])
                w_qk.append(t)
                t = wpool.tile([128, C], bf16, tag=f"wv{ci}")
                nc.sync.dma_start(t[:], wqkv[ci * 128:(ci + 1) * 128,
                                             2 * C:3 * C])
                w_v.append(t)
                t = wpool.tile([128, C], bf16, tag=f"wp{ci}")
                nc.sync.dma_start(t[:], wproj[ci * 128:(ci + 1) * 128, :])
                w_p.append(t)

            bqk = wpool.tile([128, 2 * C // 128], f32, tag="bqk")
            nc.sync.dma_start(
                bqk[:], bqkv[0:2 * C].rearrange("(m p) -> p m", p=128))
            bv_row = wpool.tile([1, C], f32, tag="bvrow")
            nc.sync.dma_start(bv_row[:],
                              bqkv[2 * C:3 * C].rearrange("(a c) -> a c", a=1))
            bv = wpool.tile([128, C], f32, tag="bv")
            nc.gpsimd.partition_broadcast(bv[:], bv_row[:])
            bp_row = wpool.tile([1, C], f32, tag="bprow")
            nc.sync.dma_start(bp_row[:],
                              bproj[:].rearrange("(a c) -> a c", a=1))
            bp = wpool.tile([128, C], f32, tag="bp")
            nc.gpsimd.partition_broadcast(bp[:], bp_row[:])

            # Persistent 2-slot K^T pads and V stationaries; constant
            # regions memset once here, steady state only writes the
            # varying regions.
            kt_slots = []   # [mt][slot] -> (tA, tB)
            for mt in range(6):
                pair = []
                for s in range(2):
                    tA = wpool.tile([128, N], bf16, tag=f"ktA{mt}_{s}")
                    tB = wpool.tile([128, N], bf16, tag=f"ktB{mt}_{s}")
                    nc.vector.memset(tA[64:128, :], 0.0)
                    nc.vector.memset(tB[0:64, :], 0.0)
                    pair.append((tA, tB))
                kt_slots.append(pair)
            v1e_slots, v1o_slots = [], []
            for tt in range(3):
                es, os_ = [], []
                for s in range(2):
                    te = wpool.tile([128, 6, HD + 1], e_dt, tag=f"v1e{tt}_{s}")
                    nc.vector.memset(te[:, :, HD:HD + 1], 1.0)
                    to = wpool.tile([128, 6, 128], e_dt, tag=f"v1o{tt}_{s}")
                    nc.vector.memset(to[:, :, 0:1], 1.0)
                    nc.vector.memset(to[:, :, 1:HD], 0.0)
                    es.append(te)
                    os_.append(to)
                v1e_slots.append(es)
                v1o_slots.append(os_)

            pv_queue = []    # (b, h) waiting for PV+normalize
            proj_queue = []  # (b, tt) waiting for output projection
            mul_queue = []   # deferred normalize multiplies

            def qk_mtile(b, mt):
                st = state[b]
                xt, off = st["xt"], st["xoff"]
                ps = pspool.tile([128, N], f32, tag="gemm",
                                 name=f"psqk{mt}_{b}")
                for ci in range(NCH):
                    nc.tensor.matmul(
                        ps[:], w_qk[ci][:, mt * 128:(mt + 1) * 128],
                        xt[ci][:, off:off + N],
                        start=(ci == 0), stop=(ci == NCH - 1))
                if mt < 6:
                    t = qkpool.tile([128, N], bf16, tag=f"qk{mt}",
                                    name=f"qk{mt}_{b}")
                    # drain on DVE (free-axis-broadcast bias) to keep ACT
                    # free for the exp stream, the per-head critical path
                    nc.vector.scalar_tensor_tensor(
                        out=t[:], in0=ps[:], scalar=1.0,
                        in1=bqk[:, mt:mt + 1].broadcast_to([128, N]),
                        op0=Alu.mult, op1=Alu.add)
                    st["qk"].append(t)
                else:
                    tA, tB = kt_slots[mt - 6][b % 2]
                    nc.scalar.activation(tA[0:64, :], ps[0:64, :],
                                         Act.Identity,
                                         bias=bqk[0:64, mt:mt + 1], scale=1.0)
                    nc.scalar.activation(tB[64:128, :], ps[64:128, :],
                                         Act.Identity,
                                         bias=bqk[64:128, mt:mt + 1],
                                         scale=1.0)
                    st["kt"].append((tA, tB))

            def v_group(b, tt, half):
                st = state[b]
                xt, off = st["xt"], st["xoff"]
                ps = pspool.tile([128, N], f32, tag="gemm",
                                 name=f"psv{tt}{half}_{b}")
                for ci in range(NCH):
                    nc.tensor.matmul(
                        ps[:], xt[ci][:, off + tt * 128:off + (tt + 1) * 128],
                        w_v[ci][:, half * N:(half + 1) * N],
                        start=(ci == 0), stop=(ci == NCH - 1))
                # psum cols = 6 heads x 64 dims; even heads -> v1e cols 0:64,
                # odd heads -> v1o cols 64:128
                ps3 = ps[:].rearrange("p (a b) -> p a b", b=128)
                bv3 = (bv[:, half * N:(half + 1) * N]
                       .rearrange("p (a b) -> p a b", b=128))
                nc.vector.scalar_tensor_tensor(
                    out=st["v1e"][tt][:, 3 * half:3 * half + 3, 0:HD],
                    in0=ps3[:, :, 0:HD], scalar=1.0, in1=bv3[:, :, 0:HD],
                    op0=Alu.mult, op1=Alu.add)
                nc.vector.scalar_tensor_tensor(
                    out=st["v1o"][tt][:, 3 * half:3 * half + 3, HD:2 * HD],
                    in0=ps3[:, :, HD:2 * HD], scalar=1.0,
                    in1=bv3[:, :, HD:2 * HD],
                    op0=Alu.mult, op1=Alu.add)

            def attn_scores(b, h):
                st = state[b]
                hp, part = divmod(h, 2)
                kt_pad = st["kt"][hp][part]
                qt_t = st["qk"][hp]
                # bank-aligned scores psum/E layout: key-chunk 0 (all 384
                # query cols) at cols 0:384, chunks 1-2 (256 search-query
                # cols each) at 512:768 and 768:1024; cols 384:512 unused.
                ps = pvpool.tile([128, 1024], f32, tag="pss",
                                 bufs=bufs["pss"], name=f"pss{h}_{b}")
                et = epool.tile([128, 1024], e_dt, tag="e", name=f"e{h}_{b}")
                nc.tensor.matmul(ps[:, 0:N], kt_pad[:, 0:128], qt_t[:, 0:N],
                                 start=True, stop=True)
                nc.tensor.matmul(ps[:, 512:768], kt_pad[:, 128:256],
                                 qt_t[:, 128:N], start=True, stop=True)
                nc.tensor.matmul(ps[:, 768:1024], kt_pad[:, 256:384],
                                 qt_t[:, 128:N], start=True, stop=True)
                nc.scalar.activation(et[:, 0:N], ps[:, 0:N],
                                     Act.Exp, bias=0.0, scale=0.125)
                nc.scalar.activation(et[:, 512:1024], ps[:, 512:1024],
                                     Act.Exp, bias=0.0, scale=0.125)
                st["e"][h] = et

            def attn_pv(b, h):
                st = state[b]
                xt2 = st["xt2"]
                et = st["e"].pop(h)
                even = h % 2 == 0
                v1 = st["v1e"] if even else st["v1o"]
                pv = pvpool.tile([128, N], f32, tag="pspv",
                                 name=f"pv{h}_{b}")
                np_ = HD + 1 if even else 128
                nc.tensor.matmul(pv[0:np_, 0:N], v1[0][:, h // 2, :],
                                 et[:, 0:N], start=True, stop=False,
                                 skip_group_check=True)
                nc.tensor.matmul(pv[0:np_, 128:N], v1[1][:, h // 2, :],
                                 et[:, 512:768], start=False, stop=False,
                                 skip_group_check=True)
                nc.tensor.matmul(pv[0:np_, 128:N], v1[2][:, h // 2, :],
                                 et[:, 768:1024], start=False, stop=True,
                                 skip_group_check=True)
                # Evacuate the psum bank immediately (cheap DVE copies) so
                # the next head's PV matmuls get a free slot in ~0.5us
                # instead of waiting out the recip->broadcast->mul chain
                # (~2.5us on HW); the normalize then runs entirely from
                # SBUF with the multiply deferred 2 heads behind the
                # broadcast (mul_queue) to avoid DVE head-of-line blocking.
                rr = rpool.tile([1, N], f32, tag="rr", name=f"rr{h}_{b}")
                xo = rpool.tile([128, N], bf16, tag="xo", name=f"xo{h}_{b}")
                if even:
                    dd = rpool.tile([1, N], f32, tag="dd", name=f"dd{h}_{b}")
                    nc.vector.tensor_copy(dd[:], pv[HD:HD + 1, :])
                    nc.vector.tensor_copy(xo[0:HD, :], pv[0:HD, :])
                    nc.vector.reciprocal_approx_fast(rr[:], dd[:])
                    brc = rpool.tile([HD, N], f32, tag="brc",
                                     name=f"brc{h}_{b}")
                    nc.gpsimd.partition_broadcast(brc[:], rr[:])
                else:
                    nc.vector.reciprocal_approx_fast(rr[:], pv[0:1, :])
                    nc.vector.tensor_copy(xo[HD:128, :], pv[HD:128, :])
                    brc = rpool.tile([128, N], f32, tag="brcf",
                                     name=f"brc{h}_{b}")
                    nc.gpsimd.partition_broadcast(brc[:], rr[:])
                mul_queue.append((b, h, xo, brc))

            def pop_mul():
                b, h, xo, brc = mul_queue.pop(0)
                xt2 = state[b]["xt2"]
                if h % 2 == 0:
                    nc.vector.tensor_mul(xt2[h // 2][0:HD, :], xo[0:HD, :],
                                         brc[:])
                else:
                    nc.vector.tensor_mul(xt2[h // 2][HD:128, :],
                                         xo[HD:128, :], brc[HD:128, :])

            def proj_group(b, tt):
                st = state[b]
                xt2 = st["xt2"]
                ot = opool.tile([128, C], f32, tag="osb", name=f"o{tt}_{b}")
                for half in range(2):
                    ps = pspool.tile([128, N], f32, tag="gemm",
                                     name=f"pso{tt}{half}_{b}")
                    for ci in range(NCH):
                        nc.tensor.matmul(
                            ps[:], xt2[ci][:, tt * 128:(tt + 1) * 128],
                            w_p[ci][:, half * N:(half + 1) * N],
                            start=(ci == 0), stop=(ci == NCH - 1))
                    nc.vector.scalar_tensor_tensor(
                        out=ot[:, half * N:(half + 1) * N], in0=ps[:],
                        scalar=1.0, in1=bp[:, half * N:(half + 1) * N],
                        op0=Alu.mult, op1=Alu.add)
                nc.sync.dma_start(
                    out[(b * 3 + tt) * 128:(b * 3 + tt + 1) * 128, :], ot[:])
                st["live"] -= 1
                if st["live"] == 0:
                    del state[b]

            def drain(pv_lag=0, proj_max=2, mul_lag=2):
                while len(pv_queue) > pv_lag:
                    attn_pv(*pv_queue.pop(0))
                    while len(mul_queue) > mul_lag:
                        pop_mul()
                while len(mul_queue) > mul_lag:
                    pop_mul()
                n = 0
                while proj_queue and n < proj_max:
                    b, tt = proj_queue[0]
                    if (any(q[0] == b for q in pv_queue)
                            or any(q[0] == b for q in mul_queue)):
                        break  # that batch's xt2 not fully emitted yet
                    proj_group(*proj_queue.pop(0))
                    n += 1

            def drain_final():
                # interleave PV chains with PE-heavy proj groups so the
                # trailing ACT/DVE latency hides under matmuls
                while pv_queue or proj_queue or mul_queue:
                    for _ in range(2):
                        if pv_queue:
                            attn_pv(*pv_queue.pop(0))
                    while len(mul_queue) > (2 if pv_queue else 0):
                        pop_mul()
                    if proj_queue:
                        b, tt = proj_queue[0]
                        if not (any(q[0] == b for q in pv_queue)
                                or any(q[0] == b for q in mul_queue)):
                            proj_group(*proj_queue.pop(0))

            xt_cache = {}  # even batch index -> pair xT tiles

            def load_xt(pb):
                xt = []
                for ci in range(NCH):
                    t = xpool.tile([128, 2 * N], bf16, tag=f"xt{ci}",
                                   name=f"xt{ci}_{pb}")
                    w = min(2 * N, tok - pb * N)
                    nc.sync.dma_start(
                        t[:, 0:w],
                        xT[ci * 128:(ci + 1) * 128, pb * N:pb * N + w])
                    xt.append(t)
                xt_cache[pb] = xt

            def fused(b):
                st = state[b] = {"qk": [], "kt": [], "e": {}, "live": 3}
                if b % 2 == 0:
                    if b not in xt_cache:
                        load_xt(b)
                    st["xt"], st["xoff"] = xt_cache[b], 0
                else:
                    st["xt"], st["xoff"] = xt_cache[b - 1], N
                    if b + 1 < nbatch:
                        # prefetch the next pair's xT a full batch early so
                        # the transfer hides under this batch's compute
                        load_xt(b + 1)
                st["v1e"] = [v1e_slots[tt][b % 2] for tt in range(3)]
                st["v1o"] = [v1o_slots[tt][b % 2] for tt in range(3)]
                st["xt2"] = [
                    xt2pool.tile([128, N], bf16, tag=f"xt2{ci}",
                                 name=f"xt2{ci}_{b}")
                    for ci in range(NCH)]
                # v-projection groups all land in hp 0-2 so that every PV
                # popped at hp>=3 (lag 6 heads) sees fully-written V
                # stationaries; output projection of the previous batch pops
                # at hp>=3, after that batch's last PV popped at hp2.
                for hp in range(6):
                    qk_mtile(b, hp)
                    qk_mtile(b, hp + 6)
                    if hp < 3:
                        v_group(b, hp, 0)
                        v_group(b, hp, 1)
                    attn_scores(b, 2 * hp)
                    pv_queue.append((b, 2 * hp))
                    attn_scores(b, 2 * hp + 1)
                    pv_queue.append((b, 2 * hp + 1))
                    drain(pv_lag=LAG_PV, proj_max=0 if hp < 3 else 2)
                proj_queue.extend((b, tt) for tt in range(3))

            loop_cm = (tc.For_i(0, loop_reps, 1) if loop_reps > 1
                       else contextlib.nullcontext())
            with loop_cm:
                for b in range(nbatch):
                    fused(b)
                drain_final()
    nc.compile()
    return nc


def _get_program():
    global _PROGRAM
    if _PROGRAM is None:
        _PROGRAM = _build_program(NB)
    return _PROGRAM


def make_in_maps(x, W_qkv, b_qkv, W_proj, b_proj):
    import ml_dtypes
    bf = ml_dtypes.bfloat16
    x = np.asarray(x, dtype=np.float32)
    W_qkv = np.asarray(W_qkv, dtype=np.float32).astype(bf)
    b_qkv = np.asarray(b_qkv, dtype=np.float32)
    W_proj = np.asarray(W_proj, dtype=np.float32).astype(bf)
    b_proj = np.asarray(b_proj, dtype=np.float32)
    in_maps = []
    for i in range(NCORES):
        xc = x[i * NB:(i + 1) * NB].reshape(TOK, C)
        in_maps.append({
            "xT": np.ascontiguousarray(xc.T).astype(bf),
            "wqkv": W_qkv, "bqkv": b_qkv,
            "wproj": W_proj, "bproj": b_proj,
        })
    return in_maps


def kernel(x, W_qkv, b_qkv, W_proj, b_proj, t_h, t_w, s_h, s_w):
    from concourse.bass_utils import run_bass_kernel_spmd

    x = np.asarray(x, dtype=np.float32)
    assert x.shape == (B, N, C)
    assert int(t_h) * int(t_w) * 2 == NT
    assert int(s_h) * int(s_w) == N - NT

    nc = _get_program()
    in_maps = make_in_maps(x, W_qkv, b_qkv, W_proj, b_proj)
    res = run_bass_kernel_spmd(nc, in_maps, core_ids=list(range(NCORES)))
    return np.concatenate(
        [r["out"].reshape(NB, N, C) for r in res.results], axis=0)
